# revision 46
# baseline (speedup 1.0000x reference)
"""Multi-head causal attention (B=4, T=2048, DM=1024, H=16, DK=DV=64) on 8 TRN2
NeuronCores.

Sharding: tensor-parallel over heads — core c owns heads {2c, 2c+1}. Each core:
  1. projects full-sequence Q^T/K^T/V^T for its 2 heads from a pre-transposed
     x^T (host supplies x^T; pure layout prep, no FLOPs),
  2. runs causal attention per (batch, head) in S^T = K Q^T layout with
     block-skipping of the fully-masked triangle; softmax denominators ride
     along as a ones-column appended to V (row 64 of the PV accumulator);
     normalization is deferred to a per-column reciprocal broadcast built with
     a K=1 PE matmul,
  3. row-shards W_o: out_partial = concat(head_out).T @ Wo[128c:128c+128] + bo/8.
Host sums the 8 partials (the W_o all-reduce done on host).
"""

import sys

for _p in ("/opt/trn_rl_repo",):
    if _p not in sys.path:
        sys.path.insert(0, _p)

import numpy as np

# ---- problem constants (hardcoded per harness contract) ----
B, T, DM = 4, 2048, 1024
H, DK = 16, 64
NCORES = 8
HL = 2                      # heads per core
SD = HL * DK                # 128: partition width of per-core head-stacked tiles
BT = B * T

# tiling
TB = 256                    # projection t-block (moving N)
PW = 1024                   # attention q "pair" width (PSUM S tile free size)
CH = 512                    # PSUM chunk / matmul moving width
ND = DM // 128              # contraction k-tiles for projections
NT = T // 128               # s-tiles per batch
NPAIR = T // PW
TBB = T // TB               # projection t-blocks per batch
SCALE = DK ** -0.5

_CACHE = {}


def _build(mode="f32"):
    import concourse.bass as bass
    import concourse.tile as tile
    from concourse import bacc, mybir

    f32 = mybir.dt.float32
    mdt = {"f32": f32, "f32r": mybir.dt.float32r,
           "bf16": mybir.dt.bfloat16}[mode]
    # bf16 denominators round the softmax scale by ~4e-3 relative — well
    # inside the 2e-2 gate, and they keep the broadcast matmul off the
    # fp32 4-cycle/row path.
    ddt = mdt
    ts = bass.ts

    def rc(ap):  # matmul operands are natively typed now
        return ap

    def dma_in(out, in_):  # SWDGE casts on the fly; HWDGE needs equal dtypes
        if out.dtype != in_.dtype:
            return nc.gpsimd.dma_start(out=out, in_=in_)
        return nc.sync.dma_start(out=out, in_=in_)

    nc = bacc.Bacc("TRN2", target_bir_lowering=False, debug=False,
                   num_devices=NCORES)

    xT = nc.dram_tensor("xT", [DM, BT], mdt, kind="ExternalInput").ap()
    wq2 = nc.dram_tensor("wq2", [DM, SD], mdt, kind="ExternalInput").ap()
    wk2 = nc.dram_tensor("wk2", [DM, SD], mdt, kind="ExternalInput").ap()
    wv2 = nc.dram_tensor("wv2", [DM, SD], mdt, kind="ExternalInput").ap()
    wo_my = nc.dram_tensor("wo_my", [SD, DM], mdt, kind="ExternalInput").ap()
    bo8 = nc.dram_tensor("bo8", [1, DM], f32, kind="ExternalInput").ap()
    outp = nc.dram_tensor("out_part", [BT, DM], mdt, kind="ExternalOutput").ap()

    # additive causal mask in S^T layout: -1e30 where q < s (strict lower)
    tri = ((1.0 - np.triu(np.ones((128, 128)))) * -1e30).astype(np.float32)
    # broadcast row lives at partition 64 so its base partition matches the
    # accumulator denominator row it multiplies (matmul requires equal bases)
    ones2 = np.zeros((65, 64), np.float32)
    ones2[64, :] = 1.0
    vones = np.ones((128, NT, 1), np.float32)
    tri_h = nc.inline_tensor(tri, name="tri_const")
    vones_h = nc.inline_tensor(vones, name="vones_const")
    ones2_h = nc.inline_tensor(ones2, name="ones2_const")

    with tile.TileContext(nc) as tc:
        with (
            tc.tile_pool(name="singles", bufs=1) as singles,
            tc.tile_pool(name="stream", bufs=3) as stream,
            tc.tile_pool(name="seq", bufs=2) as seq,
            tc.tile_pool(name="att", bufs=4) as att,
            tc.tile_pool(name="small", bufs=4) as small,
            tc.tile_pool(name="pmm", bufs=2, space="PSUM") as pmm,
            tc.tile_pool(name="pacc", bufs=4, space="PSUM") as pacc,
        ):
            # ---- constants into SBUF ----
            xT_r0 = xT.rearrange("(a p) t -> p a t", p=128)
            # the batch-0 first x-tile heads the DMA queue: it gates the
            # very first matmul together with wq
            xts0 = stream.tile([128, ND, TB], mdt, tag="xts", name="xts0")
            nc.sync.dma_start(out=xts0, in_=xT_r0[:, :, 0:TB])
            # projection weights next: they gate the first matmuls, while
            # tri/ones2/wo aren't needed until attention/Wo
            w_sb = {}
            for name, src in (("q", wq2), ("k", wk2), ("v", wv2)):
                w_sb[name] = singles.tile([128, ND, SD], mdt, tag=f"w{name}", name=f"w{name}_sb")
                dma_in(
                    w_sb[name],
                    src.rearrange("(a p) m -> p a m", p=128),
                )

            # tiles allocated now, DMAs emitted after proj(0)'s so the first
            # x-tile load isn't queued behind constants that aren't needed
            # until attention/Wo
            tri_sb = singles.tile([128, 128], f32, tag="tri")
            ones2_sb = singles.tile([65, 64], ddt, tag="ones2")
            wo_sb = singles.tile([128, DM], mdt, tag="wo")

            def load_late_consts():
                nc.sync.dma_start(out=tri_sb, in_=tri_h.ap())
                dma_in(ones2_sb, ones2_h.ap())
                dma_in(wo_sb, wo_my)

            xT_r = xT.rearrange("(a p) t -> p a t", p=128)

            tiles = {}

            def emit_proj(b, flush=None):
                # ================= phase A: projections for batch b ========
                # q/k: weight-stationary, x moving -> q^T/k^T [dk2, t].
                # v: x-stationary, Wv moving -> V directly in [t, v2] layout
                # (no PE transposes); ones columns DMA'd separately.
                qt = seq.tile([128, T], mdt, tag="qt")
                kt = seq.tile([128, T], mdt, tag="kt")
                vsb = seq.tile([128, NT * 130], mdt, tag="vsb")
                tiles[b] = (qt, kt, vsb)
                vsb3 = vsb.rearrange("p (n c) -> p n c", c=130)
                dma_in(vsb3[:, :, 64:65], vones_h.ap())
                dma_in(vsb3[:, :, 129:130], vones_h.ap())
                for i in range(TBB):
                    if b == 0 and i == 0:
                        xts = xts0      # preloaded ahead of the weights
                    else:
                        xts = stream.tile([128, ND, TB], mdt, tag="xts")
                        dma_in(
                            xts,
                            xT_r[:, :, b * T + i * TB: b * T + (i + 1) * TB],
                        )
                    for name, dst in (("q", qt), ("k", kt)):
                        pj = pmm.tile([128, TB], f32, tag="s")
                        for a in range(ND):
                            nc.tensor.matmul(
                                pj,
                                rc(w_sb[name][:, a, :]),
                                rc(xts[:, a, :]),
                                start=(a == 0),
                                stop=(a == ND - 1),
                            )
                        nc.scalar.copy(dst[:, ts(i, TB)], pj)
                    for half in (0, 1):
                        j = 2 * i + half          # 128-wide t-tile index
                        pv = pmm.tile([128, 128], f32, tag="s", name="pv")
                        for a in range(ND):
                            nc.tensor.matmul(
                                pv,
                                rc(xts[:, a, ts(half, 128)]),
                                rc(w_sb["v"][:, a, :]),
                                start=(a == 0),
                                stop=(a == ND - 1),
                            )
                        # scatter [t,128] -> [t, v0|_, v1|_] (cols 0-63, 65-128)
                        pv3 = pv.rearrange("p (h v) -> p h v", h=2)
                        nc.vector.tensor_copy(
                            vsb3[:, j, :].rearrange(
                                "p (h c) -> p h c", h=2)[:, :, 0:64],
                            pv3,
                        )
                    if i == 0 and flush is not None:
                        # previous batch's last-pair normalize: deferred past
                        # this block's matmuls so the PE isn't stalled on the
                        # d-row copies at the batch boundary
                        flush()

            def emit_attention(b, head_fill=None):
                # ================= attention for batch b ===================
                # h1's normalized rows are produced at base partition 0 (DVE
                # lanes are partition-locked) and DMA'd to partitions 64..127
                # of onorm at the end of the batch.
                qt, kt, vsb = tiles[b]
                onorm = seq.tile([128, T], mdt, tag="onorm")
                onorm1 = seq.tile([64, T], mdt, tag="onorm1")
                tiles[b] = tiles[b] + (onorm,)
                pending_norm = [None]
                for p in range(NPAIR):
                    nj = p * (PW // 128) + (PW // 128)       # j in [0, nj)
                    acc = [[pacc.tile([65, CH], f32, tag="acc", name="acc")
                            for _ in range(PW // CH)] for _ in (0, 1)]
                    for j in range(nj):
                        j_rel = j - p * (PW // 128)
                        c0 = max(0, 128 * j_rel)             # first valid col
                        qq_lo = c0 // CH                     # first chunk kept
                        # chunk boundaries (PSUM bank limit: 512 f32 out)
                        chunks = []
                        col = c0
                        while col < PW:
                            hi = min(PW, (col // CH + 1) * CH)
                            chunks.append((col, hi))
                            col = hi
                        # QK for both heads, chunk-outer/head-inner: the
                        # h0/h1 matmuls land in row groups 0/64 (from the
                        # kt slice base partition).
                        S = [pmm.tile([128, PW], f32, tag="s", name="s0"),
                             pmm.tile([128, PW], f32, tag="s", name="s1")]
                        for (lo, hi) in chunks:
                            for h in (0, 1):
                                nc.tensor.matmul(
                                    S[h][:, lo:hi],
                                    rc(kt[h * 64:(h + 1) * 64, ts(j, 128)]),
                                    rc(qt[h * 64:(h + 1) * 64,
                                          p * PW + lo: p * PW + hi]),
                                    start=True, stop=True,
                                )
                        if j_rel >= 0:           # diagonal: mask on PSUM
                            for h in (0, 1):
                                nc.vector.tensor_add(
                                    S[h][:, c0:c0 + 128], S[h][:, c0:c0 + 128],
                                    tri_sb,
                                )
                        E = [att.tile([128, PW], mdt, tag="expt", name="e0"),
                             att.tile([128, PW], mdt, tag="expt", name="e1")]
                        for h in (0, 1):
                            nc.scalar.activation(
                                out=E[h][:, c0:PW], in_=S[h][:, c0:PW],
                                func=mybir.ActivationFunctionType.Exp,
                                scale=SCALE,
                            )
                        if j == 0 and pending_norm[0] is not None:
                            # previous pair's normalize, deferred past this
                            # pair's first QK/exp so the PE doesn't stall on
                            # the DVE d-row copies at the pair boundary
                            pending_norm[0]()
                            pending_norm[0] = None
                        if j == 0 and p == 0 and head_fill is not None:
                            # tail of the previous batch's Wo: ready PE work
                            # to cover the first exp of this batch
                            head_fill()
                        for h in (0, 1):
                            for qq in range(qq_lo, PW // CH):
                                last = nj - 1 if qq > 0 else \
                                    min(nj - 1, p * (PW // 128) + 3)
                                lo = max(c0, qq * CH)
                                nc.tensor.matmul(
                                    acc[h][qq][:, lo - qq * CH: CH],
                                    rc(vsb[:, j * 130 + h * 65:
                                           j * 130 + h * 65 + 65]),
                                    rc(E[h][:, lo:(qq + 1) * CH]),
                                    start=(j == 0), stop=(j == last),
                                    skip_group_check=True,
                                )
                    # normalize: O' rows 0..63 per head, denominator row 64.
                    # reciprocal straight off the PSUM d-row (DVE), broadcast
                    # to 64 partitions on GpSimd (keeps the PE out of the
                    # chain), then scale O' straight out of PSUM.
                    # normalize: O' rows 0..63 per head, denominator row 64.
                    # d-row -> SBUF (aligned copy at partition 64), broadcast
                    # to 64 partitions with a K=1 matmul, fast reciprocal,
                    # then scale O' straight out of PSUM. Emission deferred
                    # into the next pair's first iteration (or flushed below
                    # for the last pair).
                    def make_norm(p, acc):
                        def norm():
                            for qq in range(PW // CH):
                                for h in (0, 1):
                                    dsb = small.tile([65, CH], ddt, tag="dsb",
                                                     name="dsb")
                                    nc.vector.tensor_copy(
                                        dsb[64:65, :], acc[h][qq][64:65, :]
                                    )
                                    dbc = pmm.tile([64, CH], f32, tag="s",
                                                   name="dbc")
                                    nc.tensor.matmul(
                                        dbc,
                                        rc(ones2_sb[64:65, :]),
                                        rc(dsb[64:65, :]),
                                        start=True, stop=True,
                                    )
                                    rcp = small.tile([64, CH], f32, tag="rcp",
                                                     name="rcp")
                                    nc.vector.reciprocal_approx_fast(
                                        out=rcp, in_=dbc)
                                    dst = onorm if h == 0 else onorm1
                                    nc.vector.tensor_mul(
                                        dst[0:64, p * PW + qq * CH:
                                            p * PW + (qq + 1) * CH],
                                        acc[h][qq][0:64, :],
                                        rcp,
                                    )
                            # place h1 rows at partitions 64..127 (DMA moves
                            # across partitions; DVE cannot)
                            nc.sync.dma_start(
                                out=onorm[64:128, p * PW:(p + 1) * PW],
                                in_=onorm1[:, p * PW:(p + 1) * PW],
                            )
                        return norm

                    pending_norm[0] = make_norm(p, acc)

                def flush():
                    if pending_norm[0] is not None:
                        pending_norm[0]()
                        pending_norm[0] = None
                return flush

            def emit_wo(b, lo=0, hi=NT):
                # ================= phase C: partial W_o for batch b ========
                onorm = tiles[b][3]
                for tc_i in range(lo, hi):
                    osb = stream.tile([128, DM], mdt, tag="osb")
                    for cc in range(DM // CH):
                        po = pmm.tile([128, CH], f32, tag="s")
                        nc.tensor.matmul(
                            po,
                            rc(onorm[:, ts(tc_i, 128)]),
                            rc(wo_sb[:, ts(cc, CH)]),
                            start=True, stop=True,
                        )
                        # bias is added host-side; plain PSUM->SBUF drain on
                        # the otherwise-idle scalar engine
                        nc.scalar.copy(osb[:, ts(cc, CH)], po)
                    r0 = b * T + tc_i * 128
                    nc.sync.dma_start(out=outp[r0:r0 + 128, :], in_=osb)

            # software pipeline: proj(b+1) sits between attention(b) and
            # Wo(b) so the in-order PE queue has independent GEMM work while
            # batch b's softmax-normalize chain (DVE/DMA) drains; the last
            # pair's normalize flushes inside proj(b+1)'s first block; two
            # Wo t-tiles trail into the next batch's attention start to
            # cover its first exp; the final batch sandwiches its last-pair
            # normalize between Wo halves (tiles 0-7 only need pair 0).
            emit_proj(0)
            load_late_consts()
            wo_tail = None
            for b in range(B):
                fl = emit_attention(b, head_fill=wo_tail)
                wo_tail = None
                if b + 1 < B:
                    emit_proj(b + 1, flush=fl)
                    emit_wo(b, 0, NT - 2)
                    wo_tail = (lambda bb: (lambda: emit_wo(bb, NT - 2, NT)))(b)
                else:
                    emit_wo(b, 0, NT // 2)
                    fl()
                    emit_wo(b, NT // 2, NT)

    nc.compile()
    return nc


MODE = "bf16"


def _get_nc():
    key = "nc" + MODE
    if key not in _CACHE:
        _CACHE[key] = _build(MODE)
    return _CACHE[key]


def _host_dt():
    if MODE == "bf16":
        import ml_dtypes
        return ml_dtypes.bfloat16
    return np.float32


def make_in_maps(x, Wq, Wk, Wv, Wo, bo):
    hdt = _host_dt()
    x2d = np.ascontiguousarray(x.reshape(BT, DM), dtype=np.float32)
    xT = np.ascontiguousarray(x2d.T).astype(hdt)
    bo8 = np.ascontiguousarray((bo / NCORES).reshape(1, DM), dtype=np.float32)
    maps = []
    for c in range(NCORES):
        h0, h1 = HL * c, HL * c + 1
        maps.append({
            "xT": xT,
            "wq2": np.ascontiguousarray(
                np.concatenate([Wq[h0], Wq[h1]], 1)).astype(hdt),
            "wk2": np.ascontiguousarray(
                np.concatenate([Wk[h0], Wk[h1]], 1)).astype(hdt),
            "wv2": np.ascontiguousarray(
                np.concatenate([Wv[h0], Wv[h1]], 1)).astype(hdt),
            "wo_my": np.ascontiguousarray(
                Wo[SD * c: SD * (c + 1)]).astype(hdt),
            "bo8": bo8,
        })
    return maps


def run(x, Wq, Wk, Wv, Wo, bo, trace=False, **spmd_kwargs):
    from concourse.bass_utils import run_bass_kernel_spmd

    nc = _get_nc()
    maps = make_in_maps(x, Wq, Wk, Wv, Wo, bo)
    res = run_bass_kernel_spmd(
        nc, maps, core_ids=list(range(NCORES)), trace=trace, **spmd_kwargs
    )
    total = np.zeros((BT, DM), np.float32)
    for r in res.results:
        total += r["out_part"].astype(np.float32)
    total += np.asarray(bo, dtype=np.float32)[None, :]
    return total.reshape(B, T, DM), res


def kernel(x, Wq, Wk, Wv, Wo, bo):
    out, _ = run(x, Wq, Wk, Wv, Wo, bo)
    return out



# revision 47
# speedup vs baseline: 1.1589x; 1.1589x over previous
"""Multi-head causal attention (B=4, T=2048, DM=1024, H=16, DK=DV=64) on 8 TRN2
NeuronCores.

Sharding: tensor-parallel over heads — core c owns heads {2c, 2c+1}. Each core:
  1. projects full-sequence Q^T/K^T/V^T for its 2 heads from a pre-transposed
     x^T (host supplies x^T; pure layout prep, no FLOPs),
  2. runs causal attention per (batch, head) in S^T = K Q^T layout with
     block-skipping of the fully-masked triangle; softmax denominators ride
     along as a ones-column appended to V (row 64 of the PV accumulator);
     normalization is deferred to a per-column reciprocal broadcast built with
     a K=1 PE matmul,
  3. row-shards W_o: out_partial = concat(head_out).T @ Wo[128c:128c+128] + bo/8.
Host sums the 8 partials (the W_o all-reduce done on host).
"""

import sys

for _p in ("/opt/trn_rl_repo",):
    if _p not in sys.path:
        sys.path.insert(0, _p)

import numpy as np

# ---- problem constants (hardcoded per harness contract) ----
B, T, DM = 4, 2048, 1024
H, DK = 16, 64
NCORES = 8
HL = 2                      # heads per core
SD = HL * DK                # 128: partition width of per-core head-stacked tiles
BT = B * T

# tiling
TB = 256                    # projection t-block (moving N)
PW = 1024                   # attention q "pair" width (PSUM S tile free size)
CH = 512                    # PSUM chunk / matmul moving width
ND = DM // 128              # contraction k-tiles for projections
NT = T // 128               # s-tiles per batch
NPAIR = T // PW
TBB = T // TB               # projection t-blocks per batch
SCALE = DK ** -0.5

_CACHE = {}


def _build(mode="f32"):
    import concourse.bass as bass
    import concourse.tile as tile
    from concourse import bacc, mybir

    f32 = mybir.dt.float32
    mdt = {"f32": f32, "f32r": mybir.dt.float32r,
           "bf16": mybir.dt.bfloat16}[mode]
    # bf16 denominators round the softmax scale by ~4e-3 relative — well
    # inside the 2e-2 gate, and they keep the broadcast matmul off the
    # fp32 4-cycle/row path.
    ddt = mdt
    ts = bass.ts

    def rc(ap):  # matmul operands are natively typed now
        return ap

    def dma_in(out, in_):  # SWDGE casts on the fly; HWDGE needs equal dtypes
        if out.dtype != in_.dtype:
            return nc.gpsimd.dma_start(out=out, in_=in_)
        return nc.sync.dma_start(out=out, in_=in_)

    nc = bacc.Bacc("TRN2", target_bir_lowering=False, debug=False,
                   num_devices=NCORES)

    xT = nc.dram_tensor("xT", [DM, BT], mdt, kind="ExternalInput").ap()
    wq2 = nc.dram_tensor("wq2", [DM, SD], mdt, kind="ExternalInput").ap()
    wk2 = nc.dram_tensor("wk2", [DM, SD], mdt, kind="ExternalInput").ap()
    wv2 = nc.dram_tensor("wv2", [DM, SD], mdt, kind="ExternalInput").ap()
    wo_my = nc.dram_tensor("wo_my", [SD, DM], mdt, kind="ExternalInput").ap()
    bo8 = nc.dram_tensor("bo8", [1, DM], f32, kind="ExternalInput").ap()
    outp = nc.dram_tensor("out_part", [BT, DM], mdt, kind="ExternalOutput").ap()

    # additive causal mask in S^T layout: -1e30 where q < s (strict lower)
    tri = ((1.0 - np.triu(np.ones((128, 128)))) * -1e30).astype(np.float32)
    # broadcast row lives at partition 64 so its base partition matches the
    # accumulator denominator row it multiplies (matmul requires equal bases)
    ones2 = np.zeros((65, 64), np.float32)
    ones2[64, :] = 1.0
    vones = np.ones((128, NT, 1), np.float32)
    tri_h = nc.inline_tensor(tri, name="tri_const")
    vones_h = nc.inline_tensor(vones, name="vones_const")
    ones2_h = nc.inline_tensor(ones2, name="ones2_const")

    with tile.TileContext(nc) as tc:
        with (
            tc.tile_pool(name="singles", bufs=1) as singles,
            tc.tile_pool(name="stream", bufs=3) as stream,
            tc.tile_pool(name="seq", bufs=2) as seq,
            tc.tile_pool(name="att", bufs=4) as att,
            tc.tile_pool(name="small", bufs=4) as small,
            tc.tile_pool(name="pmm", bufs=2, space="PSUM") as pmm,
            tc.tile_pool(name="pacc", bufs=4, space="PSUM") as pacc,
        ):
            # ---- constants into SBUF ----
            xT_r0 = xT.rearrange("(a p) t -> p a t", p=128)
            # the batch-0 first x-tile heads the DMA queue: it gates the
            # very first matmul together with wq
            xts0 = stream.tile([128, ND, TB], mdt, tag="xts", name="xts0")
            nc.sync.dma_start(out=xts0, in_=xT_r0[:, :, 0:TB])
            # projection weights next: they gate the first matmuls, while
            # tri/ones2/wo aren't needed until attention/Wo
            w_sb = {}
            for name, src in (("q", wq2), ("k", wk2), ("v", wv2)):
                w_sb[name] = singles.tile([128, ND, SD], mdt, tag=f"w{name}", name=f"w{name}_sb")
                dma_in(
                    w_sb[name],
                    src.rearrange("(a p) m -> p a m", p=128),
                )

            # tiles allocated now, DMAs emitted after proj(0)'s so the first
            # x-tile load isn't queued behind constants that aren't needed
            # until attention/Wo
            tri_sb = singles.tile([128, 128], f32, tag="tri")
            ones2_sb = singles.tile([65, 64], ddt, tag="ones2")
            wo_sb = singles.tile([128, DM], mdt, tag="wo")

            def load_late_consts():
                nc.sync.dma_start(out=tri_sb, in_=tri_h.ap())
                dma_in(ones2_sb, ones2_h.ap())
                dma_in(wo_sb, wo_my)

            xT_r = xT.rearrange("(a p) t -> p a t", p=128)

            tiles = {}

            def emit_proj(b, flush=None):
                # ================= phase A: projections for batch b ========
                # q/k: weight-stationary, x moving -> q^T/k^T [dk2, t].
                # v: x-stationary, Wv moving -> V directly in [t, v2] layout
                # (no PE transposes); ones columns DMA'd separately.
                qt = seq.tile([128, T], mdt, tag="qt")
                kt = seq.tile([128, T], mdt, tag="kt")
                vsb = seq.tile([128, NT * 130], mdt, tag="vsb")
                tiles[b] = (qt, kt, vsb)
                vsb3 = vsb.rearrange("p (n c) -> p n c", c=130)
                dma_in(vsb3[:, :, 64:65], vones_h.ap())
                dma_in(vsb3[:, :, 129:130], vones_h.ap())
                for i in range(TBB):
                    if b == 0 and i == 0:
                        xts = xts0      # preloaded ahead of the weights
                    else:
                        xts = stream.tile([128, ND, TB], mdt, tag="xts")
                        dma_in(
                            xts,
                            xT_r[:, :, b * T + i * TB: b * T + (i + 1) * TB],
                        )
                    for name, dst in (("q", qt), ("k", kt)):
                        pj = pmm.tile([128, TB], f32, tag="s")
                        for a in range(ND):
                            nc.tensor.matmul(
                                pj,
                                rc(w_sb[name][:, a, :]),
                                rc(xts[:, a, :]),
                                start=(a == 0),
                                stop=(a == ND - 1),
                            )
                        nc.scalar.copy(dst[:, ts(i, TB)], pj)
                    for half in (0, 1):
                        j = 2 * i + half          # 128-wide t-tile index
                        pv = pmm.tile([128, 128], f32, tag="s", name="pv")
                        for a in range(ND):
                            nc.tensor.matmul(
                                pv,
                                rc(xts[:, a, ts(half, 128)]),
                                rc(w_sb["v"][:, a, :]),
                                start=(a == 0),
                                stop=(a == ND - 1),
                            )
                        # scatter [t,128] -> [t, v0|_, v1|_] (cols 0-63, 65-128)
                        pv3 = pv.rearrange("p (h v) -> p h v", h=2)
                        nc.vector.tensor_copy(
                            vsb3[:, j, :].rearrange(
                                "p (h c) -> p h c", h=2)[:, :, 0:64],
                            pv3,
                        )
                    if i == 0 and flush is not None:
                        # previous batch's last-pair normalize: deferred past
                        # this block's matmuls so the PE isn't stalled on the
                        # d-row copies at the batch boundary
                        flush()

            def emit_attention(b):
                # ================= attention for batch b ===================
                # h1's normalized rows are produced at base partition 0 (DVE
                # lanes are partition-locked) and DMA'd to partitions 64..127
                # of onorm at the end of the batch.
                qt, kt, vsb = tiles[b]
                onorm = seq.tile([128, T], mdt, tag="onorm")
                onorm1 = seq.tile([64, T], mdt, tag="onorm1")
                tiles[b] = tiles[b] + (onorm,)
                pending_norm = [None]
                for p in range(NPAIR):
                    nj = p * (PW // 128) + (PW // 128)       # j in [0, nj)
                    acc = [[pacc.tile([65, CH], f32, tag="acc", name="acc")
                            for _ in range(PW // CH)] for _ in (0, 1)]
                    for j in range(nj):
                        j_rel = j - p * (PW // 128)
                        c0 = max(0, 128 * j_rel)             # first valid col
                        qq_lo = c0 // CH                     # first chunk kept
                        # chunk boundaries (PSUM bank limit: 512 f32 out)
                        chunks = []
                        col = c0
                        while col < PW:
                            hi = min(PW, (col // CH + 1) * CH)
                            chunks.append((col, hi))
                            col = hi
                        # QK for both heads, chunk-outer/head-inner: the
                        # h0/h1 matmuls land in row groups 0/64 (from the
                        # kt slice base partition).
                        S = [pmm.tile([128, PW], f32, tag="s", name="s0"),
                             pmm.tile([128, PW], f32, tag="s", name="s1")]
                        for (lo, hi) in chunks:
                            for h in (0, 1):
                                nc.tensor.matmul(
                                    S[h][:, lo:hi],
                                    rc(kt[h * 64:(h + 1) * 64, ts(j, 128)]),
                                    rc(qt[h * 64:(h + 1) * 64,
                                          p * PW + lo: p * PW + hi]),
                                    start=True, stop=True,
                                )
                        if j_rel >= 0:           # diagonal: mask on PSUM
                            for h in (0, 1):
                                nc.vector.tensor_add(
                                    S[h][:, c0:c0 + 128], S[h][:, c0:c0 + 128],
                                    tri_sb,
                                )
                        E = [att.tile([128, PW], mdt, tag="expt", name="e0"),
                             att.tile([128, PW], mdt, tag="expt", name="e1")]
                        for h in (0, 1):
                            nc.scalar.activation(
                                out=E[h][:, c0:PW], in_=S[h][:, c0:PW],
                                func=mybir.ActivationFunctionType.Exp,
                                scale=SCALE,
                            )
                        if j == 0 and pending_norm[0] is not None:
                            # previous pair's normalize, deferred past this
                            # pair's first QK/exp so the PE doesn't stall on
                            # the DVE d-row copies at the pair boundary
                            pending_norm[0]()
                            pending_norm[0] = None
                        for h in (0, 1):
                            for qq in range(qq_lo, PW // CH):
                                last = nj - 1 if qq > 0 else \
                                    min(nj - 1, p * (PW // 128) + 3)
                                lo = max(c0, qq * CH)
                                nc.tensor.matmul(
                                    acc[h][qq][:, lo - qq * CH: CH],
                                    rc(vsb[:, j * 130 + h * 65:
                                           j * 130 + h * 65 + 65]),
                                    rc(E[h][:, lo:(qq + 1) * CH]),
                                    start=(j == 0), stop=(j == last),
                                    skip_group_check=True,
                                )
                    # normalize: O' rows 0..63 per head, denominator row 64.
                    # reciprocal straight off the PSUM d-row (DVE), broadcast
                    # to 64 partitions on GpSimd (keeps the PE out of the
                    # chain), then scale O' straight out of PSUM.
                    # normalize: O' rows 0..63 per head, denominator row 64.
                    # d-row -> SBUF (aligned copy at partition 64), broadcast
                    # to 64 partitions with a K=1 matmul, fast reciprocal,
                    # then scale O' straight out of PSUM. Emission deferred
                    # into the next pair's first iteration (or flushed below
                    # for the last pair).
                    def make_norm(p, acc):
                        def norm():
                            for qq in range(PW // CH):
                                for h in (0, 1):
                                    dsb = small.tile([65, CH], ddt, tag="dsb",
                                                     name="dsb")
                                    nc.vector.tensor_copy(
                                        dsb[64:65, :], acc[h][qq][64:65, :]
                                    )
                                    dbc = pmm.tile([64, CH], f32, tag="s",
                                                   name="dbc")
                                    nc.tensor.matmul(
                                        dbc,
                                        rc(ones2_sb[64:65, :]),
                                        rc(dsb[64:65, :]),
                                        start=True, stop=True,
                                    )
                                    rcp = small.tile([64, CH], f32, tag="rcp",
                                                     name="rcp")
                                    nc.vector.reciprocal_approx_fast(
                                        out=rcp, in_=dbc)
                                    dst = onorm if h == 0 else onorm1
                                    nc.vector.tensor_mul(
                                        dst[0:64, p * PW + qq * CH:
                                            p * PW + (qq + 1) * CH],
                                        acc[h][qq][0:64, :],
                                        rcp,
                                    )
                            # place h1 rows at partitions 64..127 (DMA moves
                            # across partitions; DVE cannot)
                            nc.sync.dma_start(
                                out=onorm[64:128, p * PW:(p + 1) * PW],
                                in_=onorm1[:, p * PW:(p + 1) * PW],
                            )
                        return norm

                    pending_norm[0] = make_norm(p, acc)

                def flush():
                    if pending_norm[0] is not None:
                        pending_norm[0]()
                        pending_norm[0] = None
                return flush

            def emit_wo(b):
                # ================= phase C: partial W_o for batch b ========
                onorm = tiles[b][3]
                for tc_i in range(NT):
                    osb = stream.tile([128, DM], mdt, tag="osb")
                    for cc in range(DM // CH):
                        po = pmm.tile([128, CH], f32, tag="s")
                        nc.tensor.matmul(
                            po,
                            rc(onorm[:, ts(tc_i, 128)]),
                            rc(wo_sb[:, ts(cc, CH)]),
                            start=True, stop=True,
                        )
                        # bias is added host-side; plain PSUM->SBUF drain on
                        # the otherwise-idle scalar engine
                        nc.scalar.copy(osb[:, ts(cc, CH)], po)
                    r0 = b * T + tc_i * 128
                    nc.sync.dma_start(out=outp[r0:r0 + 128, :], in_=osb)

            # software pipeline: proj(b+1) sits between attention(b) and
            # Wo(b) so the in-order PE queue has independent GEMM work while
            # batch b's softmax-normalize chain (DVE/DMA) drains; the last
            # pair's normalize flushes inside proj(b+1)'s first block.
            emit_proj(0)
            load_late_consts()
            for b in range(B):
                fl = emit_attention(b)
                if b + 1 < B:
                    emit_proj(b + 1, flush=fl)
                else:
                    fl()
                emit_wo(b)
                del tiles[b]

    nc.compile()
    return nc


MODE = "bf16"


def _get_nc():
    key = "nc" + MODE
    if key not in _CACHE:
        _CACHE[key] = _build(MODE)
    return _CACHE[key]


def _host_dt():
    if MODE == "bf16":
        import ml_dtypes
        return ml_dtypes.bfloat16
    return np.float32


def make_in_maps(x, Wq, Wk, Wv, Wo, bo):
    hdt = _host_dt()
    x2d = np.ascontiguousarray(x.reshape(BT, DM), dtype=np.float32)
    xT = np.ascontiguousarray(x2d.T).astype(hdt)
    bo8 = np.ascontiguousarray((bo / NCORES).reshape(1, DM), dtype=np.float32)
    maps = []
    for c in range(NCORES):
        h0, h1 = HL * c, HL * c + 1
        maps.append({
            "xT": xT,
            "wq2": np.ascontiguousarray(
                np.concatenate([Wq[h0], Wq[h1]], 1)).astype(hdt),
            "wk2": np.ascontiguousarray(
                np.concatenate([Wk[h0], Wk[h1]], 1)).astype(hdt),
            "wv2": np.ascontiguousarray(
                np.concatenate([Wv[h0], Wv[h1]], 1)).astype(hdt),
            "wo_my": np.ascontiguousarray(
                Wo[SD * c: SD * (c + 1)]).astype(hdt),
            "bo8": bo8,
        })
    return maps


def run(x, Wq, Wk, Wv, Wo, bo, trace=False, **spmd_kwargs):
    from concourse.bass_utils import run_bass_kernel_spmd

    nc = _get_nc()
    maps = make_in_maps(x, Wq, Wk, Wv, Wo, bo)
    res = run_bass_kernel_spmd(
        nc, maps, core_ids=list(range(NCORES)), trace=trace, **spmd_kwargs
    )
    total = np.zeros((BT, DM), np.float32)
    for r in res.results:
        total += r["out_part"].astype(np.float32)
    total += np.asarray(bo, dtype=np.float32)[None, :]
    return total.reshape(B, T, DM), res


def kernel(x, Wq, Wk, Wv, Wo, bo):
    out, _ = run(x, Wq, Wk, Wv, Wo, bo)
    return out

